# revision 13
# baseline (speedup 1.0000x reference)
"""FAST multi-head attention (p=2 Taylor linear attention) for Trainium2.

Self-contained: accepts FULL inputs q,k,v [2,16,4096,32] fp32, returns the
full output [2,16,4096,32]. Shards the 32 (b,h) pairs across 8 NeuronCores
(4 per core), one Bass/Tile kernel run SPMD via PJRT.

Per (b,h) (A0=1, A1=1, A2=0.5):
  num[n,e'] = sum_m v'[m,e'] * (A0 + A1 (q.k_m) + A2 (q.k_m)^2),  v' = [v | 1]
  out       = num[:, :32] / num[:, 32]
factorized through quadratic features with a cyclic pair cover
(gap = 16..1, descending):
  k-side:  ktile[m, 33+32p+d] = k_d * k_{(d+16-p)%32}  (one DVE op per
           128-row tile, negative-stride output), diag k^2 at cols 545:577
  kv:      kvt_A = v'^T [ones|k|cover_hi], kvt_B = v'^T [cover_lo|diag] (PE)
  q-side:  PhiOff^T = Square(E^T qT) (PE + ScalarE), PhiD^T = A2 qT^2
  ansT    = wc^T PhiOff^T + wd^T PhiD^T + wq^T qT + bias  (PE, fp32r)
The device stores ansT [33, N] (numerator rows 0:32, denominator row 32);
the divide + final [e,n]->[n,e] transpose happen on the host.
Spurious diagonal terms from the cover are cancelled through the wd weights
(wd = KV2dd - Hmat @ KVc).
"""
import dataclasses
import numpy as np

import concourse.bass as bass
import concourse.tile as tile
from concourse import mybir, bacc
from concourse.bass_utils import run_bass_kernel_spmd

F32 = mybir.dt.float32
BF16 = mybir.dt.bfloat16
A0, A1, A2 = 1.0, 1.0, 0.5
B, H, N, D = 2, 16, 4096, 32
NJ = 16                    # cover gaps, stored descending 16..1
F = NJ * D                 # 512 off-diagonal features
NCORES = 8
BH_PER_CORE = (B * H) // NCORES   # 4
NT = N // 128              # 32 n-tiles per (b,h)
E1 = D + 2                 # 34 output rows: 32 num + den + dup-den pad


def _host_consts():
    E = np.zeros((D, F), np.float32)
    Hm = np.zeros((D, F), np.float32)
    for jj in range(NJ):          # block jj holds gap = 16 - jj
        gap = NJ - jj
        beta2 = A2 if gap < 16 else A2 / 2.0
        beta = np.sqrt(beta2).astype(np.float32)
        c = beta2 / A2
        for d1 in range(D):
            f = jj * D + d1
            d2 = (d1 + gap) % D
            E[d1, f] += beta
            E[d2, f] += beta
            Hm[d1, f] += c
            Hm[d2, f] += c
    import ml_dtypes
    E4 = np.tile(E, (4, 1)).astype(ml_dtypes.bfloat16)       # [128, 512]
    HmT = np.ascontiguousarray(
        Hm.T.reshape(4, 128, D).transpose(1, 0, 2)).astype(ml_dtypes.bfloat16)
    ident = np.eye(128, dtype=np.float32)
    return E4, HmT, ident


def _ap_free(x: bass.AP, free_ap, extra_offset=0):
    return dataclasses.replace(
        x, offset=x.offset + extra_offset, ap=[x.ap[0]] + [list(p) for p in free_ap]
    )


def build_nc():
    nc = bacc.Bacc(None, target_bir_lowering=False)
    R32 = mybir.dt.float32r

    def r(ap):
        return ap if ap.dtype == R32 else ap.bitcast(R32)

    def tr(out_ap, in_ap, ident_ap, tile_position=None):
        nc.tensor.matmul(out_ap, in_ap, ident_ap, is_transpose=True,
                         tile_position=tile_position, skip_group_check=True)

    qin = nc.declare_dram_parameter("qin", [BH_PER_CORE, N, D], BF16, isOutput=False)
    kin = nc.declare_dram_parameter("kin", [BH_PER_CORE, N, D], BF16, isOutput=False)
    vin = nc.declare_dram_parameter("vin", [BH_PER_CORE, N, D], BF16, isOutput=False)
    e4_in = nc.declare_dram_parameter("e4", [128, F], BF16, isOutput=False)
    hmt_in = nc.declare_dram_parameter("hmt", [128, 4, D], BF16, isOutput=False)
    id_in = nc.declare_dram_parameter("ident", [128, 128], F32, isOutput=False)
    out = nc.declare_dram_parameter("out", [BH_PER_CORE, E1, N], F32, isOutput=True)

    SQ = mybir.ActivationFunctionType.Square
    sqrt_a2 = float(np.sqrt(A2))

    with tile.TileContext(nc) as tc:
        with (
            tc.tile_pool(name="sb_const", bufs=1) as sb_const,
            tc.tile_pool(name="sb_q", bufs=2) as sb_q,
            tc.tile_pool(name="sb_k", bufs=3) as sb_k,
            tc.tile_pool(name="sb_w", bufs=2) as sb_w,
            tc.tile_pool(name="sb_phi", bufs=3) as sb_phi,
            tc.tile_pool(name="sb_ep", bufs=3) as sb_ep,
            tc.tile_pool(name="ps_kv", bufs=1, space="PSUM") as ps_kv,
            tc.tile_pool(name="ps_u", bufs=3, space="PSUM") as ps_u,
            tc.tile_pool(name="ps_ans", bufs=2, space="PSUM") as ps_ans,
            tc.tile_pool(name="ps_sm", bufs=1, space="PSUM") as ps_sm,
        ):
            e4 = sb_const.tile([128, F], BF16)
            nc.sync.dma_start(out=e4[:], in_=e4_in[:])
            hmt = sb_const.tile([128, 4, D], BF16)
            nc.sync.dma_start(out=hmt[:], in_=hmt_in[:])
            ident = sb_const.tile([128, 128], F32)
            nc.sync.dma_start(out=ident[:], in_=id_in[:])
            identb = sb_const.tile([128, 128], BF16)
            nc.vector.tensor_copy(identb[:], ident[:])

            kts = [sb_const.tile([128, 8, 578], BF16, name=f"kts{i}")
                   for i in range(3)]
            vxs = [sb_const.tile([128, 8, 34], BF16, name=f"vxs{i}")
                   for i in range(3)]
            for i in range(3):
                nc.gpsimd.memset(kts[i][:, :, 0:2], 1.0)
                nc.gpsimd.memset(vxs[i][:, :, 32:34], 1.0)
            rot = [0]

            for b in range(BH_PER_CORE):
                qv = qin[b].rearrange("(a bb p) d -> p bb a d", a=4, bb=8)
                kv_ = kin[b].rearrange("(t p) d -> p t d", p=128)
                vv = vin[b].rearrange("(t p) d -> p t d", p=128)

                # ---------- q loads + transposes ------------------------------
                q_sb = sb_q.tile([128, 8, 4, D], BF16, tag="q_sb")
                for a in range(4):
                    nc.sync.dma_start(out=q_sb[:, :, a, :], in_=qv[:, :, a, :])
                qtb = sb_q.tile([128, 8, 128], BF16, tag="qtb")
                for bb in range(8):
                    qt_ps = ps_sm.tile([128, 128], BF16, tag="sm")
                    tr(qt_ps[:], q_sb[:, bb, :, :], identb[:])
                    nc.scalar.copy(out=qtb[:, bb, :], in_=qt_ps[:])
                phidt = sb_q.tile([128, 8, 128], BF16, tag="phidt")
                nc.scalar.activation(out=phidt[:], in_=qtb[:], func=SQ,
                                     scale=sqrt_a2)

                # ---------- phase 1: k features + KV accumulation -------------
                kvt_a = ps_kv.tile([E1, 290], F32, tag="kvt_a")
                kvt_b = ps_kv.tile([E1, 288], F32, tag="kvt_b")
                for g in range(4):
                    ts8 = slice(8 * g, 8 * g + 8)
                    stg = sb_k.tile([128, 8, 48], BF16, tag="stg")
                    nc.sync.dma_start(out=stg[:, :, 0:32], in_=kv_[:, ts8, :])
                    nc.sync.dma_start(out=stg[:, :, 32:48],
                                      in_=kv_[:, ts8, 0:16])
                    kt = kts[rot[0] % 3]
                    vx = vxs[rot[0] % 3]
                    rot[0] += 1
                    nc.sync.dma_start(out=kt[:, :, 2:34], in_=kv_[:, ts8, :])
                    nc.sync.dma_start(out=vx[:, :, 0:32], in_=vv[:, ts8, :])
                    for tt in range(8):
                        t = 8 * g + tt
                        kbase = stg[:, tt, 0:32]
                        in0 = _ap_free(kbase, [[0, NJ + 1], [1, D]])
                        in1 = _ap_free(kbase, [[1, NJ + 1], [1, D]])
                        # out col for gap j at 545-32j (diag j=0 at 545:577)
                        dst = _ap_free(kt[:, tt, 546:547],
                                       [[-D, NJ + 1], [1, D]])
                        nc.vector.tensor_mul(dst, in0, in1)
                        lhs = vx[:, tt, :]
                        st, sp = (t == 0), (t == NT - 1)
                        nc.tensor.matmul(kvt_a[:], lhs, kt[:, tt, 0:290],
                                         start=st, stop=sp)
                        nc.tensor.matmul(kvt_b[:], lhs, kt[:, tt, 290:578],
                                         start=st, stop=sp)

                # ---------- phase 2: weight assembly --------------------------
                a_sb = sb_w.tile([E1, 290], F32, tag="a_sb")
                nc.vector.tensor_copy(a_sb[:], kvt_a[:])
                b_sb = sb_w.tile([E1, 288], F32, tag="b_sb")
                nc.vector.tensor_copy(b_sb[:], kvt_b[:])

                # wc: cover features (gap 16..1), 4 groups of 128
                wc = sb_w.tile([128, 4, E1], BF16, tag="wc")
                cov = [a_sb[:, 34:162], a_sb[:, 162:290],
                       b_sb[:, 0:128], b_sb[:, 128:256]]
                for s in range(4):
                    trc = ps_sm.tile([128, E1], F32, tag="sm")
                    tr(trc[0:128, :], cov[s], ident[0:E1, 0:E1])
                    nc.scalar.copy(out=wc[:, s, :], in_=trc[0:128, :])

                # HKVcT [E1, 32] = (Hmat @ KVc)^T
                hk = ps_sm.tile([E1, D], F32, tag="sm")
                for s in range(4):
                    nc.tensor.matmul(hk[:], wc[:, s, :], hmt[:, s, :],
                                     start=(s == 0), stop=(s == 3))

                # wDT [E1, 32] = KV2ddT - HKVcT
                wdt = sb_w.tile([E1, D], F32, tag="wdt")
                nc.vector.scalar_tensor_tensor(
                    out=wdt[:], in0=b_sb[:, 256:288], scalar=1.0, in1=hk[:],
                    op0=mybir.AluOpType.mult, op1=mybir.AluOpType.subtract,
                )

                # wq4/wd4: transpose to [32, E1], replicate to 4 groups
                wq4 = sb_w.tile([128, E1], BF16, tag="wq4")
                wd4 = sb_w.tile([128, E1], BF16, tag="wd4")
                trq = ps_sm.tile([128, E1], F32, tag="sm")
                trd = ps_sm.tile([128, E1], F32, tag="sm")
                tr(trq[0:32, :], a_sb[:, 2:34], ident[0:E1, 0:E1])
                tr(trd[0:32, :], wdt[:], ident[0:E1, 0:E1])
                nc.scalar.copy(out=wq4[0:32, :], in_=trq[0:32, :])
                nc.scalar.copy(out=wd4[0:32, :], in_=trd[0:32, :])
                for a in range(1, 4):
                    nc.sync.dma_start(out=wq4[32 * a:32 * a + 32, :],
                                      in_=wq4[0:32, :])
                    nc.sync.dma_start(out=wd4[32 * a:32 * a + 32, :],
                                      in_=wd4[0:32, :])

                # ---------- phase 3: paired 512-wide n-chunks -----------------
                for half in range(2):
                    blk = slice(4 * half, 4 * half + 4)
                    for g2 in range(2):
                        pair = (2 * g2, 2 * g2 + 1)
                        phit = [sb_phi.tile([128, 4, 512], BF16, tag="phit",
                                            name=f"phit{ci}")
                                for ci in range(2)]
                        for s in range(4):
                            for ci, a in enumerate(pair):
                                pa = slice(32 * a, 32 * a + 32)
                                u_ps = ps_u.tile([128, 512], F32, tag="u")
                                nc.tensor.matmul(
                                    u_ps[:],
                                    e4[pa, 128 * s:128 * (s + 1)],
                                    qtb[pa, blk, :],
                                    tile_position=(32 * a, 0))
                                nc.scalar.activation(
                                    out=phit[ci][:, s, :], in_=u_ps[:],
                                    func=SQ, scale=1.0)
                        for ci, a in enumerate(pair):
                            pa = slice(32 * a, 32 * a + 32)
                            ansT = ps_ans.tile([E1, 512], F32, tag="ansT")
                            nc.tensor.matmul(ansT[:], wq4[pa, :],
                                             qtb[pa, blk, :],
                                             start=True, stop=False,
                                             tile_position=(32 * a, 0))
                            nc.tensor.matmul(ansT[:], wd4[pa, :],
                                             phidt[pa, blk, :],
                                             start=False, stop=False,
                                             tile_position=(32 * a, 0))
                            for s in range(4):
                                nc.tensor.matmul(ansT[:], wc[:, s, :],
                                                 phit[ci][:, s, :],
                                                 start=False, stop=(s == 3))
                            anssb = sb_ep.tile([E1, 512], F32, tag="anssb")
                            nc.vector.tensor_scalar_add(anssb[:], ansT[:],
                                                        a_sb[:, 0:1])
                            off = 1024 * a + 512 * half
                            nc.sync.dma_start(out=out[b][:, off:off + 512],
                                              in_=anssb[:])

    nc.compile()
    return nc


_NC_CACHE = None


def _get_nc():
    global _NC_CACHE
    if _NC_CACHE is None:
        _NC_CACHE = build_nc()
    return _NC_CACHE


def _in_maps(q, k, v):
    import ml_dtypes
    qf = q.reshape(B * H, N, D).astype(ml_dtypes.bfloat16)
    kf = k.reshape(B * H, N, D).astype(ml_dtypes.bfloat16)
    vf = v.reshape(B * H, N, D).astype(ml_dtypes.bfloat16)
    E4, HmT, ident = _host_consts()
    in_maps = []
    for c in range(NCORES):
        sl = slice(c * BH_PER_CORE, (c + 1) * BH_PER_CORE)
        in_maps.append({
            "qin": np.ascontiguousarray(qf[sl]),
            "kin": np.ascontiguousarray(kf[sl]),
            "vin": np.ascontiguousarray(vf[sl]),
            "e4": E4, "hmt": HmT, "ident": ident,
        })
    return in_maps


def _postprocess(res):
    outs = [res.results[c]["out"] for c in range(NCORES)]
    allt = np.concatenate(outs, axis=0).reshape(B, H, E1, N)
    num = allt[:, :, 0:D, :]
    den = allt[:, :, D:D + 1, :]
    return np.ascontiguousarray(
        (num / den).transpose(0, 1, 3, 2)).astype(np.float32)


def run_traced(q, k, v):
    q = np.ascontiguousarray(np.asarray(q, dtype=np.float32))
    k = np.ascontiguousarray(np.asarray(k, dtype=np.float32))
    v = np.ascontiguousarray(np.asarray(v, dtype=np.float32))
    nc = _get_nc()
    try:
        return run_bass_kernel_spmd(nc, _in_maps(q, k, v),
                                    core_ids=list(range(NCORES)), trace=True)
    except Exception as e:
        print("traced run failed:", e)
        return None


def kernel(q, k, v):
    q = np.ascontiguousarray(np.asarray(q, dtype=np.float32))
    k = np.ascontiguousarray(np.asarray(k, dtype=np.float32))
    v = np.ascontiguousarray(np.asarray(v, dtype=np.float32))
    assert q.shape == (B, H, N, D)
    nc = _get_nc()
    res = run_bass_kernel_spmd(nc, _in_maps(q, k, v),
                               core_ids=list(range(NCORES)))
    return _postprocess(res)


if __name__ == "__main__":
    rng = np.random.default_rng(0)
    q = rng.standard_normal((B, H, N, D), dtype=np.float32)
    k = rng.standard_normal((B, H, N, D), dtype=np.float32)
    v = rng.standard_normal((B, H, N, D), dtype=np.float32)
    o = kernel(q, k, v)
    print("ran", o.shape, o.dtype)


# revision 14
# speedup vs baseline: 1.1293x; 1.1293x over previous
"""FAST multi-head attention (p=2 Taylor linear attention) for Trainium2.

Self-contained: accepts FULL inputs q,k,v [2,16,4096,32] fp32, returns the
full output [2,16,4096,32]. Shards the 32 (b,h) pairs across 8 NeuronCores
(4 per core), one Bass/Tile kernel run SPMD via PJRT.

Per (b,h) (A0=1, A1=1, A2=0.5):
  num[n,e'] = sum_m v'[m,e'] * (A0 + A1 (q.k_m) + A2 (q.k_m)^2),  v' = [v | 1]
  out       = num[:, :32] / num[:, 32]
factorized through quadratic features with a cyclic pair cover
(gap = 16..1, descending):
  k-side:  ktile[m, 33+32p+d] = k_d * k_{(d+16-p)%32}  (one DVE op per
           128-row tile, negative-stride output), diag k^2 at cols 545:577
  kv:      kvt_A = v'^T [ones|k|cover_hi], kvt_B = v'^T [cover_lo|diag] (PE)
  q-side:  PhiOff^T = Square(E^T qT) (PE + ScalarE), PhiD^T = A2 qT^2
  ansT    = wc^T PhiOff^T + wd^T PhiD^T + wq^T qT + bias  (PE, fp32r)
The device stores ansT [33, N] (numerator rows 0:32, denominator row 32);
the divide + final [e,n]->[n,e] transpose happen on the host.
Spurious diagonal terms from the cover are cancelled through the wd weights
(wd = KV2dd - Hmat @ KVc).
"""
import dataclasses
import numpy as np

import concourse.bass as bass
import concourse.tile as tile
from concourse import mybir, bacc
from concourse.bass_utils import run_bass_kernel_spmd

F32 = mybir.dt.float32
BF16 = mybir.dt.bfloat16
A0, A1, A2 = 1.0, 1.0, 0.5
B, H, N, D = 2, 16, 4096, 32
NJ = 16                    # cover gaps, stored descending 16..1
F = NJ * D                 # 512 off-diagonal features
NCORES = 8
BH_PER_CORE = (B * H) // NCORES   # 4
NT = N // 128              # 32 n-tiles per (b,h)
E1 = D + 2                 # 34 output rows: 32 num + den + dup-den pad


def _host_consts():
    E = np.zeros((D, F), np.float32)
    Hm = np.zeros((D, F), np.float32)
    for jj in range(NJ):          # block jj holds gap = 16 - jj
        gap = NJ - jj
        beta2 = A2 if gap < 16 else A2 / 2.0
        beta = np.sqrt(beta2).astype(np.float32)
        c = beta2 / A2
        for d1 in range(D):
            f = jj * D + d1
            d2 = (d1 + gap) % D
            E[d1, f] += beta
            E[d2, f] += beta
            Hm[d1, f] += c
            Hm[d2, f] += c
    import ml_dtypes
    E4 = np.tile(E, (4, 1)).astype(ml_dtypes.bfloat16)       # [128, 512]
    HmT = np.ascontiguousarray(
        Hm.T.reshape(4, 128, D).transpose(1, 0, 2)).astype(ml_dtypes.bfloat16)
    ident = np.eye(128, dtype=np.float32)
    return E4, HmT, ident


def _ap_free(x: bass.AP, free_ap, extra_offset=0):
    return dataclasses.replace(
        x, offset=x.offset + extra_offset, ap=[x.ap[0]] + [list(p) for p in free_ap]
    )


def build_nc():
    nc = bacc.Bacc(None, target_bir_lowering=False)
    R32 = mybir.dt.float32r

    def r(ap):
        return ap if ap.dtype == R32 else ap.bitcast(R32)

    def tr(out_ap, in_ap, ident_ap, tile_position=None):
        nc.tensor.matmul(out_ap, in_ap, ident_ap, is_transpose=True,
                         tile_position=tile_position, skip_group_check=True)

    qin = nc.declare_dram_parameter("qin", [BH_PER_CORE, N, D], BF16, isOutput=False)
    kin = nc.declare_dram_parameter("kin", [BH_PER_CORE, N, D], BF16, isOutput=False)
    vin = nc.declare_dram_parameter("vin", [BH_PER_CORE, N, D], BF16, isOutput=False)
    e4_in = nc.declare_dram_parameter("e4", [128, F], BF16, isOutput=False)
    hmt_in = nc.declare_dram_parameter("hmt", [128, 4, D], BF16, isOutput=False)
    id_in = nc.declare_dram_parameter("ident", [128, 128], F32, isOutput=False)
    out = nc.declare_dram_parameter("out", [BH_PER_CORE, E1, N], F32, isOutput=True)

    SQ = mybir.ActivationFunctionType.Square
    sqrt_a2 = float(np.sqrt(A2))

    with tile.TileContext(nc) as tc:
        with (
            tc.tile_pool(name="sb_const", bufs=1) as sb_const,
            tc.tile_pool(name="sb_q", bufs=2) as sb_q,
            tc.tile_pool(name="sb_k", bufs=3) as sb_k,
            tc.tile_pool(name="sb_w", bufs=2) as sb_w,
            tc.tile_pool(name="sb_phi", bufs=4) as sb_phi,
            tc.tile_pool(name="sb_ep", bufs=3) as sb_ep,
            tc.tile_pool(name="ps_kv", bufs=1, space="PSUM") as ps_kv,
            tc.tile_pool(name="ps_u", bufs=3, space="PSUM") as ps_u,
            tc.tile_pool(name="ps_ans", bufs=2, space="PSUM") as ps_ans,
            tc.tile_pool(name="ps_sm", bufs=1, space="PSUM") as ps_sm,
        ):
            e4 = sb_const.tile([128, F], BF16)
            nc.sync.dma_start(out=e4[:], in_=e4_in[:])
            hmt = sb_const.tile([128, 4, D], BF16)
            nc.sync.dma_start(out=hmt[:], in_=hmt_in[:])
            ident = sb_const.tile([128, 128], F32)
            nc.sync.dma_start(out=ident[:], in_=id_in[:])
            identb = sb_const.tile([128, 128], BF16)
            nc.vector.tensor_copy(identb[:], ident[:])

            kts = [sb_const.tile([128, 8, 578], BF16, name=f"kts{i}")
                   for i in range(3)]
            vxs = [sb_const.tile([128, 8, 34], BF16, name=f"vxs{i}")
                   for i in range(3)]
            for i in range(3):
                nc.gpsimd.memset(kts[i][:, :, 0:2], 1.0)
                nc.gpsimd.memset(vxs[i][:, :, 32:34], 1.0)
            rot = [0]

            def emit_qload(b):
                qv = qin[b].rearrange("(a bb p) d -> p bb a d", a=4, bb=8)
                q_sb = sb_q.tile([128, 8, 4, D], BF16, tag="q_sb",
                                 name=f"q_sb{b}")
                for a in range(4):
                    nc.sync.dma_start(out=q_sb[:, :, a, :], in_=qv[:, :, a, :])
                qtb = sb_q.tile([128, 8, 128], BF16, tag="qtb",
                                name=f"qtb{b}")
                for bb in range(8):
                    qt_ps = ps_sm.tile([128, 128], BF16, tag="sm",
                                       name=f"qt_ps{b}_{bb}")
                    tr(qt_ps[:], q_sb[:, bb, :, :], identb[:])
                    nc.scalar.copy(out=qtb[:, bb, :], in_=qt_ps[:])
                phidt = sb_q.tile([128, 8, 128], BF16, tag="phidt",
                                  name=f"phidt{b}")
                nc.scalar.activation(out=phidt[:], in_=qtb[:], func=SQ,
                                     scale=sqrt_a2)
                return qtb, phidt

            def emit_p1_group(cur, g):
                b = cur["b"]
                kv_ = kin[b].rearrange("(t p) d -> p t d", p=128)
                vv = vin[b].rearrange("(t p) d -> p t d", p=128)
                ts8 = slice(8 * g, 8 * g + 8)
                stg = sb_k.tile([128, 8, 48], BF16, tag="stg",
                                name=f"stg{b}_{g}")
                nc.sync.dma_start(out=stg[:, :, 0:32], in_=kv_[:, ts8, :])
                nc.sync.dma_start(out=stg[:, :, 32:48],
                                  in_=kv_[:, ts8, 0:16])
                kt = kts[rot[0] % 3]
                vx = vxs[rot[0] % 3]
                rot[0] += 1
                nc.sync.dma_start(out=kt[:, :, 2:34], in_=kv_[:, ts8, :])
                nc.sync.dma_start(out=vx[:, :, 0:32], in_=vv[:, ts8, :])
                for tt in range(8):
                    t = 8 * g + tt
                    kbase = stg[:, tt, 0:32]
                    in0 = _ap_free(kbase, [[0, NJ + 1], [1, D]])
                    in1 = _ap_free(kbase, [[1, NJ + 1], [1, D]])
                    # out col for gap j at 546-32j (diag j=0 at 546:578)
                    dst = _ap_free(kt[:, tt, 546:547],
                                   [[-D, NJ + 1], [1, D]])
                    nc.vector.tensor_mul(dst, in0, in1)
                    lhs = vx[:, tt, :]
                    st, sp = (t == 0), (t == NT - 1)
                    nc.tensor.matmul(cur["ka"][:], lhs, kt[:, tt, 0:290],
                                     start=st, stop=sp)
                    nc.tensor.matmul(cur["kb"][:], lhs, kt[:, tt, 290:578],
                                     start=st, stop=sp)

            def emit_p2(cur):
                b = cur["b"]
                a_sb = sb_w.tile([E1, 290], F32, tag="a_sb", name=f"a_sb{b}")
                nc.vector.tensor_copy(a_sb[:], cur["ka"][:])
                b_sb = sb_w.tile([E1, 288], F32, tag="b_sb", name=f"b_sb{b}")
                nc.vector.tensor_copy(b_sb[:], cur["kb"][:])

                wc = sb_w.tile([128, 4, E1], BF16, tag="wc", name=f"wc{b}")
                cov = [a_sb[:, 34:162], a_sb[:, 162:290],
                       b_sb[:, 0:128], b_sb[:, 128:256]]
                for s in range(4):
                    trc = ps_sm.tile([128, E1], F32, tag="sm",
                                     name=f"trc{b}_{s}")
                    tr(trc[0:128, :], cov[s], ident[0:E1, 0:E1])
                    nc.scalar.copy(out=wc[:, s, :], in_=trc[0:128, :])

                hk = ps_sm.tile([E1, D], F32, tag="sm", name=f"hk{b}")
                for s in range(4):
                    nc.tensor.matmul(hk[:], wc[:, s, :], hmt[:, s, :],
                                     start=(s == 0), stop=(s == 3))

                wdt = sb_w.tile([E1, D], F32, tag="wdt", name=f"wdt{b}")
                nc.vector.scalar_tensor_tensor(
                    out=wdt[:], in0=b_sb[:, 256:288], scalar=1.0, in1=hk[:],
                    op0=mybir.AluOpType.mult, op1=mybir.AluOpType.subtract,
                )

                wq4 = sb_w.tile([128, E1], BF16, tag="wq4", name=f"wq4{b}")
                wd4 = sb_w.tile([128, E1], BF16, tag="wd4", name=f"wd4{b}")
                trq = ps_sm.tile([128, E1], F32, tag="sm", name=f"trq{b}")
                trd = ps_sm.tile([128, E1], F32, tag="sm", name=f"trd{b}")
                tr(trq[0:32, :], a_sb[:, 2:34], ident[0:E1, 0:E1])
                tr(trd[0:32, :], wdt[:], ident[0:E1, 0:E1])
                nc.scalar.copy(out=wq4[0:32, :], in_=trq[0:32, :])
                nc.scalar.copy(out=wd4[0:32, :], in_=trd[0:32, :])
                for a in range(1, 4):
                    nc.sync.dma_start(out=wq4[32 * a:32 * a + 32, :],
                                      in_=wq4[0:32, :])
                    nc.sync.dma_start(out=wd4[32 * a:32 * a + 32, :],
                                      in_=wd4[0:32, :])
                cur["a_sb"] = a_sb
                cur["wc"] = wc
                cur["wq4"] = wq4
                cur["wd4"] = wd4

            def emit_p3_pair(prev, half, g2):
                b = prev["b"]
                qtb, phidt = prev["qtb"], prev["phidt"]
                wc, wq4, wd4 = prev["wc"], prev["wq4"], prev["wd4"]
                blk = slice(4 * half, 4 * half + 4)
                pair = (2 * g2, 2 * g2 + 1)
                phit = [sb_phi.tile([128, 4, 512], BF16, tag="phit",
                                    name=f"phit{b}_{half}_{g2}_{ci}")
                        for ci in range(2)]
                for s in range(4):
                    for ci, a in enumerate(pair):
                        pa = slice(32 * a, 32 * a + 32)
                        u_ps = ps_u.tile([128, 512], F32, tag="u",
                                         name=f"u{b}_{half}_{g2}_{s}_{ci}")
                        nc.tensor.matmul(
                            u_ps[:],
                            e4[pa, 128 * s:128 * (s + 1)],
                            qtb[pa, blk, :],
                            tile_position=(32 * a, 0))
                        nc.scalar.activation(
                            out=phit[ci][:, s, :], in_=u_ps[:],
                            func=SQ, scale=1.0)
                for ci, a in enumerate(pair):
                    pa = slice(32 * a, 32 * a + 32)
                    ansT = ps_ans.tile([E1, 512], F32, tag="ansT",
                                       name=f"ansT{b}_{half}_{g2}_{ci}")
                    nc.tensor.matmul(ansT[:], wq4[pa, :],
                                     qtb[pa, blk, :],
                                     start=True, stop=False,
                                     tile_position=(32 * a, 0))
                    nc.tensor.matmul(ansT[:], wd4[pa, :],
                                     phidt[pa, blk, :],
                                     start=False, stop=False,
                                     tile_position=(32 * a, 0))
                    for s in range(4):
                        nc.tensor.matmul(ansT[:], wc[:, s, :],
                                         phit[ci][:, s, :],
                                         start=False, stop=(s == 3))
                    anssb = sb_ep.tile([E1, 512], F32, tag="anssb",
                                       name=f"anssb{b}_{half}_{g2}_{ci}")
                    nc.vector.tensor_scalar_add(anssb[:], ansT[:],
                                                prev["a_sb"][:, 0:1])
                    off = 1024 * a + 512 * half
                    nc.sync.dma_start(out=out[b][:, off:off + 512],
                                      in_=anssb[:])

            # software pipeline: phase-3 of bh-1 interleaves with phase-1 of bh
            prev = None
            for b in range(BH_PER_CORE + 1):
                cur = None
                if b < BH_PER_CORE:
                    qtb, phidt = emit_qload(b)
                    ka = ps_kv.tile([E1, 290], F32, tag="kvt_a",
                                    name=f"kvt_a{b}")
                    kb = ps_kv.tile([E1, 288], F32, tag="kvt_b",
                                    name=f"kvt_b{b}")
                    cur = dict(b=b, qtb=qtb, phidt=phidt, ka=ka, kb=kb)
                for i in range(4):
                    if prev is not None:
                        emit_p3_pair(prev, i // 2, i % 2)
                    if cur is not None:
                        emit_p1_group(cur, i)
                if cur is not None:
                    emit_p2(cur)
                prev = cur

    nc.compile()
    return nc


_NC_CACHE = None


def _get_nc():
    global _NC_CACHE
    if _NC_CACHE is None:
        _NC_CACHE = build_nc()
    return _NC_CACHE


def _in_maps(q, k, v):
    import ml_dtypes
    qf = q.reshape(B * H, N, D).astype(ml_dtypes.bfloat16)
    kf = k.reshape(B * H, N, D).astype(ml_dtypes.bfloat16)
    vf = v.reshape(B * H, N, D).astype(ml_dtypes.bfloat16)
    E4, HmT, ident = _host_consts()
    in_maps = []
    for c in range(NCORES):
        sl = slice(c * BH_PER_CORE, (c + 1) * BH_PER_CORE)
        in_maps.append({
            "qin": np.ascontiguousarray(qf[sl]),
            "kin": np.ascontiguousarray(kf[sl]),
            "vin": np.ascontiguousarray(vf[sl]),
            "e4": E4, "hmt": HmT, "ident": ident,
        })
    return in_maps


def _postprocess(res):
    outs = [res.results[c]["out"] for c in range(NCORES)]
    allt = np.concatenate(outs, axis=0).reshape(B, H, E1, N)
    num = allt[:, :, 0:D, :]
    den = allt[:, :, D:D + 1, :]
    return np.ascontiguousarray(
        (num / den).transpose(0, 1, 3, 2)).astype(np.float32)


def run_traced(q, k, v):
    q = np.ascontiguousarray(np.asarray(q, dtype=np.float32))
    k = np.ascontiguousarray(np.asarray(k, dtype=np.float32))
    v = np.ascontiguousarray(np.asarray(v, dtype=np.float32))
    nc = _get_nc()
    try:
        return run_bass_kernel_spmd(nc, _in_maps(q, k, v),
                                    core_ids=list(range(NCORES)), trace=True)
    except Exception as e:
        print("traced run failed:", e)
        return None


def kernel(q, k, v):
    q = np.ascontiguousarray(np.asarray(q, dtype=np.float32))
    k = np.ascontiguousarray(np.asarray(k, dtype=np.float32))
    v = np.ascontiguousarray(np.asarray(v, dtype=np.float32))
    assert q.shape == (B, H, N, D)
    nc = _get_nc()
    res = run_bass_kernel_spmd(nc, _in_maps(q, k, v),
                               core_ids=list(range(NCORES)))
    return _postprocess(res)


if __name__ == "__main__":
    rng = np.random.default_rng(0)
    q = rng.standard_normal((B, H, N, D), dtype=np.float32)
    k = rng.standard_normal((B, H, N, D), dtype=np.float32)
    v = rng.standard_normal((B, H, N, D), dtype=np.float32)
    o = kernel(q, k, v)
    print("ran", o.shape, o.dtype)
